# revision 8
# baseline (speedup 1.0000x reference)
"""Trainium2 Bass kernel for nn_MultiHeadAttention_23467701305746.

Reference computation (batch 8, seq 1024, hidden 512, 16 heads x 32):
  q/k/v = relu(x @ W + b); scores = q k^T / sqrt(32); attn = softmax(scores)
  out = attn @ v + x;  BatchNorm1d over (batch, seq) per channel, eps=1e-3.

Sharding: data-parallel over batch, 1 batch element per NeuronCore (8 cores).
BatchNorm batch statistics are combined with a tiny (4 KB) AllReduce.

The kernel is softmax-exp bound: 16.8M exp elements/core = 131072/lane.
ACT runs exp at 1 elem/cycle/lane (1.2 GHz), so exp alone would be ~110us.
v2 splits exp across TWO engines working on alternating score chunks:
  - ACT: true exp (PSUM f32 -> SBUF bf16), (FD+222)/1.2 ns per chunk.
  - DVE: one-op Schraudolph: bf16_bits = round_i16(s * A7 + B7) with
    A7 = log2(e)/sqrt(32)*128, B7 ~ 16249.5. The f32->i16 output convert
    rounds to nearest (hardware-verified), so a single tensor_scalar
    (mult, add) from PSUM produces the bf16 exp approximation directly.
    Elementwise error +-3.3%; softmax ratio cancellation keeps the final
    output error ~3e-4 (verified in numpy bit-exact sim).
Remaining elementwise work balances: relu epilogues / U evacuations split
ACT/DVE by knobs; residual add, square (BN variance), and the final BN
scale/shift run on GPSIMD; weight f32->bf16 casts ride on gpsimd SWDGE
DMA (descriptor cast). Per head pair (2 heads row/col-packed per PE pass):
  S^T[k,q] = kT_h^T qT_h (row-tiled x2); exp chunk [128,2,512] -> bf16
  One 33-col matmul per head gives U^T rows AND the softmax denominator
  row in a single pass (stationary = [v_h | 1]).
U^T chunks --PE bf16 transpose--> layout A; strided reciprocal + broadcast
multiply normalizes; residual add + BN partial sums (ones-matmul)
-> AllReduce -> scale/shift (Quake rsqrt on DVE; no ACT table swap) -> out.
"""

import math
import numpy as np
from contextlib import ExitStack

import concourse.bass as bass
import concourse.tile as tile
from concourse import bacc, mybir
from concourse.bass_utils import run_bass_kernel_spmd
from concourse.masks import make_identity

F32 = mybir.dt.float32
F32R = mybir.dt.float32r
BF16 = mybir.dt.bfloat16
I32 = mybir.dt.int32
I16 = mybir.dt.int16
OP = mybir.AluOpType
AF = mybir.ActivationFunctionType

N_CORES = 8
S = 1024          # sequence length per core (= per batch element)
H = 512           # hidden
NH = 16           # heads
D = 32            # head dim
KC = H // 128     # 4 contraction chunks over hidden
SC = S // 128     # 8 chunks over sequence
QH = S // 512     # 2 query halves (N=512 fp32 matmul limit)
BN_EPS = 1e-3
INV_SQRT_D = 1.0 / math.sqrt(D)
N_ROWS_TOTAL = 8 * S  # BN stats denominator (batch*seq)
QUAKE_C = 0x5F3759DF

# Schraudolph exp constants (bf16 bit pattern = round(s*A7 + B7))
EXP_A7 = float(math.log2(math.e) / math.sqrt(D) * 128.0)
EXP_B7 = 16249.5

# ---- balance knobs ----
EXP_ACT_NUM = 62      # score chunks of 128 assigned to ACT (rest DVE)
XT_EVAC_ACT = 4       # of the 8 xT evac chunks, how many on ACT (rest DVE)
US_EVAC_ACT = 8       # of the 8 U evacuations, how many on ACT (rest DVE)
V_RELU_DVE = False    # v-projection relu epilogue on DVE instead of ACT
RESID_GPS = True      # residual add on GPSIMD
SQ_GPS = True         # BN square on GPSIMD
T2_GPS = True         # BN scale/shift on GPSIMD
CONVW_GPS_DMA = True  # weight f32->bf16 converts via gpsimd dma cast


def _exp_engine(chunk_idx):
    """Deterministic ACT/DVE interleave with EXP_ACT_NUM/128 chunks on ACT,
    spread evenly so the engines alternate rather than phase-separate."""
    pos = chunk_idx % 128
    acc_before = (pos * EXP_ACT_NUM) // 128
    acc_after = ((pos + 1) * EXP_ACT_NUM) // 128
    return "act" if acc_after > acc_before else "dve"


def emit_body(nc, tc, outer_ctx, tens, with_tail=True, parts=("prep", "attn")):
    x, wq, bq, wk, bk, wv, bv, gamma, beta, out = tens
    ctx = outer_ctx.enter_context(ExitStack())

    const = ctx.enter_context(tc.tile_pool(name="const", bufs=1))
    xpool = ctx.enter_context(tc.tile_pool(name="xpool", bufs=1))
    qkp = ctx.enter_context(tc.tile_pool(name="qkp", bufs=1))
    vpool = ctx.enter_context(tc.tile_pool(name="vpool", bufs=1))
    statp = ctx.enter_context(tc.tile_pool(name="statp", bufs=1))
    psum = ctx.enter_context(tc.tile_pool(name="psum", bufs=1, space="PSUM"))
    psum2 = ctx.enter_context(tc.tile_pool(name="psum2", bufs=2, space="PSUM"))

    # ---- constants ----
    ident = const.tile([128, 128], F32, tag="ident", name="ident")
    make_identity(nc, ident[:, :])
    ident_bf = const.tile([128, 128], BF16, tag="ident_bf", name="ident_bf")
    nc.vector.tensor_copy(ident_bf[:, :], ident[:, :])
    ones_f32 = const.tile([128, 512], F32, tag="ones_f32", name="ones_f32")
    nc.vector.memset(ones_f32[:, :], 1.0)
    exp_warm = const.tile([1, 1], F32, tag="exp_warm", name="exp_warm")
    nc.scalar.activation(exp_warm[:, :], ones_f32[0:1, 0:1], AF.Exp)
    warm_mv = const.tile([128, 512], BF16, tag="warm_mv", name="warm_mv")
    nc.vector.memset(warm_mv[:, :], 0.0)
    wps = psum2.tile([128, 512], F32, tag="tps", name="tps")
    for _ in range(10):
        nc.tensor.matmul(wps[:, :], ident_bf[:, :], warm_mv[:, :],
                         start=True, stop=True)
    ones_row_b = const.tile([1, 128], BF16, tag="ones_row_b", name="ones_row_b")  # K=1 lhsT
    nc.vector.tensor_copy(ones_row_b[:, :], ones_f32[0:1, 0:128])
    ones_row_r = const.tile([1, 128], F32R, tag="ones_row_r", name="ones_row_r")
    nc.vector.tensor_copy(ones_row_r[:, :], ones_f32[0:1, 0:128])
    ones_col_r = const.tile([128, 1], F32R, tag="ones_col_r", name="ones_col_r")
    nc.vector.tensor_copy(ones_col_r[:, :], ones_f32[:, 0:1])
    gamma_sb = const.tile([1, 512], F32, tag="gamma", name="gamma")
    beta_sb = const.tile([1, 512], F32, tag="beta", name="beta")

    # ---- x load (strided DMAs so transposes start early) ----
    x_sb = xpool.tile([128, SC, 512], F32, tag="x_sb", name="x_sb")
    xr = x[:, :].rearrange("(r p) c -> p r c", p=128)
    for q4 in range(4):
        lo, hi = q4 * (SC // 4), (q4 + 1) * (SC // 4)
        nc.sync.dma_start(x_sb[:, lo:hi, :], xr[:, lo:hi, :])

    qT = [qkp.tile([128, S], BF16, tag=f"qT{c}", name=f"qT{c}") for c in range(KC)]
    kT = [qkp.tile([128, S], BF16, tag=f"kT{c}", name=f"kT{c}") for c in range(KC)]
    # v is stored as [128k, head, 33]: columns 0:32 are v_h, column 32 is 1.0
    # so one U matmul per head yields U rows plus the softmax denominator row
    v_r = [vpool.tile([128, NH, 33], BF16, tag=f"v{kv}", name=f"v{kv}") for kv in range(SC)]

    if "prep" not in parts:
        nc.sync.dma_start(gamma_sb[:, :], gamma[:].unsqueeze(0))
        nc.sync.dma_start(beta_sb[:, :], beta[:].unsqueeze(0))
        # bisection mode: fill qT/kT/v with memset instead of real projections
        for c in range(KC):
            nc.vector.memset(qT[c][:, :], 0.01)
            nc.vector.memset(kT[c][:, :], 0.01)
        for kv in range(SC):
            nc.vector.memset(v_r[kv][:, :, :], 0.5)
            nc.vector.tensor_copy(v_r[kv][:, :, 32:33], ones_f32[:, 0:NH].unsqueeze(2))
    # ---- pools (prep + attention live together for the interleave) ----
    expp = ctx.enter_context(tc.tile_pool(name="expp", bufs=2))
    usb = ctx.enter_context(tc.tile_pool(name="usb", bufs=2))
    outp = ctx.enter_context(tc.tile_pool(name="outp", bufs=1))
    tmpp = ctx.enter_context(tc.tile_pool(name="tmpp", bufs=2))
    outA = outp.tile([128, SC, 512], F32R, tag="outA", name="outA")

    have_prep = "prep" in parts
    have_attn = "attn" in parts
    exp_chunk_counter = [0]

    if have_prep:
        wpool = ctx.enter_context(tc.tile_pool(name="wpool", bufs=1))
        wstg = ctx.enter_context(tc.tile_pool(name="wstg", bufs=1))
        xtp = ctx.enter_context(tc.tile_pool(name="xtp", bufs=1))

        # one strided DMA per weight matrix into f32 staging [128p, kc, 512c]
        # (p = contraction sub-index), then one cast to bf16
        w_st, w_r = {}, {}
        for name, wt in (("q", wq), ("k", wk), ("v", wv)):
            st = wstg.tile([128, KC, 512], F32, tag=f"ws{name}", name=f"ws{name}")
            nc.sync.dma_start(st[:, :, :],
                              wt[:, :].rearrange("(k p) c -> p k c", p=128))
            w_st[name] = st
            w_r[name] = wpool.tile([128, KC, 512], BF16, tag=f"w{name}",
                                   name=f"w{name}")
        bqk = wpool.tile([128, 2, KC], F32, tag="bqk", name="bqk")
        nc.sync.dma_start(bqk[:, 0, :], bq[:].rearrange("(k p) -> p k", p=128))
        nc.sync.dma_start(bqk[:, 1, :], bk[:].rearrange("(k p) -> p k", p=128))
        bv_sb = wpool.tile([1, 512], F32, tag="bv_sb", name="bv_sb")
        nc.sync.dma_start(bv_sb[:, :], bv[:].unsqueeze(0))
        nc.sync.dma_start(gamma_sb[:, :], gamma[:].unsqueeze(0))
        nc.sync.dma_start(beta_sb[:, :], beta[:].unsqueeze(0))
        bv_b = wpool.tile([1, 512], BF16, tag="bv_b", name="bv_b")
        nc.vector.tensor_copy(bv_b[:, :], bv_sb[:, :])

        def conv_w(name):
            if CONVW_GPS_DMA:
                nc.gpsimd.dma_start(
                    w_r[name][:, :, :].rearrange("p a b -> p (a b)"),
                    w_st[name][:, :, :].rearrange("p a b -> p (a b)"))
            else:
                nc.vector.tensor_copy(
                    w_r[name][:, :, :].rearrange("p a b -> p (a b)"),
                    w_st[name][:, :, :].rearrange("p a b -> p (a b)"))

        # q/k weight converts first (they gate the first projections)
        conv_w("q")
        conv_w("k")

        # x transpose -> xT (bf16); PSUM evacuation split ACT/DVE
        xT = [xtp.tile([128, S], BF16, tag=f"xT{c}", name=f"xT{c}") for c in range(KC)]
        for c in range(KC):
            for half in range(2):
                tp4 = psum2.tile([128, 4, 128], F32, tag="tps", name="tps")
                for rr in range(4):
                    r = half * 4 + rr
                    nc.tensor.transpose(tp4[:, rr, :],
                                        x_sb[:, r, c * 128:(c + 1) * 128],
                                        ident[:, :])
                dst = xT[c][:, half * 512:(half + 1) * 512]
                src = tp4[:, :, :].rearrange("p a b -> p (a b)")
                if c * 2 + half < XT_EVAC_ACT:
                    nc.scalar.copy(dst, src)
                else:
                    nc.vector.tensor_copy(dst, src)

        def proj_qk(oc):
            # projections q,k chunk oc (layout B); relu+bias fused on ACT
            for wi, (wkey, dest) in enumerate((("q", qT), ("k", kT))):
                for sh in range(QH):
                    ps = psum2.tile([128, 512], F32, tag="tps", name="tps")
                    for kc in range(KC):
                        nc.tensor.matmul(
                            ps[:, :],
                            w_r[wkey][:, kc, oc * 128:(oc + 1) * 128],
                            xT[kc][:, sh * 512:(sh + 1) * 512],
                            start=(kc == 0), stop=(kc == KC - 1))
                    nc.scalar.activation(
                        dest[oc][:, sh * 512:(sh + 1) * 512],
                        ps[:, :], AF.Relu, bias=bqk[:, wi, oc:oc + 1],
                        scale=1.0)

        def proj_v():
            # v (layout A); bias via ones-matmul, relu on ACT or DVE
            conv_w("v")
            for kv in range(SC):
                ps = psum2.tile([128, 512], F32, tag="tps", name="tps")
                for kc in range(KC):
                    nc.tensor.matmul(ps[:, :], xT[kc][:, kv * 128:(kv + 1) * 128],
                                     w_r["v"][:, kc, :],
                                     start=(kc == 0), stop=False)
                nc.tensor.matmul(ps[:, :], ones_row_b[:, :], bv_b[:, :],
                                 start=False, stop=True)
                dst = v_r[kv][:, :, 0:32]
                src = ps[:, :].rearrange("p (h d) -> p h d", h=NH)
                if V_RELU_DVE:
                    nc.vector.tensor_scalar(out=dst, in0=src, scalar1=0.0,
                                            scalar2=None, op0=OP.max)
                else:
                    nc.scalar.activation(dst, src, AF.Relu)
                nc.vector.tensor_copy(v_r[kv][:, :, 32:33], ones_f32[:, 0:NH].unsqueeze(2))

    def emit_pair_mm(hp):
        h0, h1 = 2 * hp, 2 * hp + 1
        ch = h0 // 4                 # qT/kT chunk holding these heads
        p0 = (h0 % 4) * 32           # partition base of h0 within chunk
        p1 = (h1 % 4) * 32
        ups = psum.tile([128, S], F32, tag="u_ps", name="u_ps")

        def emit_scores(kc, qh):
            # half-size chunk with a double-buffered PSUM tag so the PE can
            # run one chunk ahead of the exp engines instead of stalling
            sps = psum.tile([128, 2, 512], F32, tag="score_ps",
                            name="score_ps", bufs=2)
            for j, pb in ((0, p0), (1, p1)):
                nc.tensor.matmul(
                    sps[:, j, :],
                    kT[ch][pb:pb + 32, kc * 128:(kc + 1) * 128],
                    qT[ch][pb:pb + 32, qh * 512:(qh + 1) * 512],
                    start=True, stop=True, tile_position=(pb, 0))
            ex = expp.tile([128, 2, 512], BF16, tag="expS", name="expS",
                           bufs=4)
            # alternate exp between ACT (true exp) and DVE (Schraudolph)
            if _exp_engine(exp_chunk_counter[0]) == "act":
                nc.scalar.activation(
                    ex[:, :, :].rearrange("p a c -> p (a c)"),
                    sps[:, :, :].rearrange("p a c -> p (a c)"),
                    AF.Exp, scale=INV_SQRT_D)
            else:
                nc.vector.tensor_scalar(
                    out=ex[:, :, :].bitcast(I16), in0=sps[:, :, :],
                    scalar1=EXP_A7, scalar2=EXP_B7,
                    op0=OP.mult, op1=OP.add)
            exp_chunk_counter[0] += 1
            return ex

        def emit_u(kc, qh, ex):
            # one 33-col matmul per head: rows 0:32 = U_h, row 32 = rowsum
            st, sp = (kc == 0), (kc == SC - 1)
            q0, q1 = qh * 512, (qh + 1) * 512
            nc.tensor.matmul(ups[0:33, q0:q1], v_r[kc][:, h0, :],
                             ex[:, 0, :], start=st, stop=sp,
                             tile_position=(0, 0))
            nc.tensor.matmul(ups[64:97, q0:q1], v_r[kc][:, h1, :],
                             ex[:, 1, :], start=st, stop=sp,
                             tile_position=(0, 64))

        chunks = [(kc, qh) for kc in range(SC) for qh in range(QH)]
        prev_c, prev_ex = chunks[0], emit_scores(*chunks[0])
        for cur in chunks[1:]:
            cur_ex = emit_scores(*cur)
            emit_u(prev_c[0], prev_c[1], prev_ex)
            prev_c, prev_ex = cur, cur_ex
        emit_u(prev_c[0], prev_c[1], prev_ex)

        # evacuate U (rows [0:32]=U_h0, [32]=rowsum_h0, [64:96]=U_h1,
        # [96]=rowsum_h1) to bf16 SBUF; transposes happen in _fin
        us = usb.tile([128, S], BF16, tag="u_sb", name="u_sb")
        if hp < US_EVAC_ACT:
            nc.scalar.copy(us[:, :], ups[:, :])
        else:
            nc.vector.tensor_copy(us[:, :], ups[:, :])
        return us

    sq = tmpp.tile([128, SC, 512], F32R, tag="sq", name="sq", bufs=1)

    def emit_pair_fin(hp, us):
        # transpose each seq chunk to layout A and divide by rowsum column;
        # all 8 bf16 transposes of the pair pack into one PSUM bank tile
        tpb = psum2.tile([128, SC, 128], BF16, tag="tps", name="tps")
        for sc in range(SC):
            nc.tensor.transpose(tpb[:, sc, :], us[:, sc * 128:(sc + 1) * 128],
                                ident_bf[:, :])
        rsr = statp.tile([128, SC, 2], F32, tag="rsr", name="rsr", bufs=2)
        # one strided reciprocal for both rowsum columns, one fused
        # broadcast-multiply for both heads (4D strided views)
        nc.vector.reciprocal(rsr[:, :, 0:2], tpb[:, :, 32:97:64])
        in0 = tpb[:, :, :].rearrange("p s (b c) -> p s b c", b=4)[:, :, 0:3:2, :]
        ov = outA[:, :, 64 * hp:64 * (hp + 1)].rearrange(
            "p s (b c) -> p s b c", b=2)
        in0b, in1b = bass.broadcast_tensor_aps(in0, rsr[:, :, 0:2].unsqueeze(3))
        nc.vector.tensor_mul(ov, in0b, in1b)
        # residual + BN square for this pair's 64 output columns, on GPSIMD
        # (otherwise idle) so the serial tail after the last pair shrinks
        c0, c1 = 64 * hp, 64 * (hp + 1)
        if RESID_GPS:
            nc.gpsimd.tensor_add(outA[:, :, c0:c1], outA[:, :, c0:c1],
                                 x_sb[:, :, c0:c1])
        else:
            nc.vector.tensor_add(outA[:, :, c0:c1], outA[:, :, c0:c1],
                                 x_sb[:, :, c0:c1])
        if SQ_GPS:
            nc.gpsimd.tensor_mul(sq[:, :, c0:c1], outA[:, :, c0:c1],
                                 outA[:, :, c0:c1])
        else:
            nc.vector.tensor_mul(sq[:, :, c0:c1], outA[:, :, c0:c1],
                                 outA[:, :, c0:c1])

    # interleaved emission: projections feed attention pairs chunk by chunk
    # so PE projection work hides under exp work (per-engine queues are
    # in-order; emission order controls overlap). Each pair's evac
    # transposes are deferred past the next projection chunk so the PE
    # doesn't idle waiting on the us-copy.
    if have_prep and have_attn:
        proj_qk(0)
        proj_v()
        usd = {}
        usd[0] = emit_pair_mm(0)
        proj_qk(1)
        usd[1] = emit_pair_mm(1)
        emit_pair_fin(0, usd[0])
        proj_qk(2)
        usd[2] = emit_pair_mm(2)
        emit_pair_fin(1, usd[1])
        proj_qk(3)
        usd[3] = emit_pair_mm(3)
        emit_pair_fin(2, usd[2])
        for hp in range(4, NH // 2):
            usd[hp] = emit_pair_mm(hp)
            emit_pair_fin(hp - 1, usd[hp - 1])
        emit_pair_fin(NH // 2 - 1, usd[NH // 2 - 1])
    elif have_prep:
        for oc in range(KC):
            proj_qk(oc)
        proj_v()
    elif have_attn:
        prev = emit_pair_mm(0)
        for hp in range(1, NH // 2):
            nxt = emit_pair_mm(hp)
            emit_pair_fin(hp - 1, prev)
            prev = nxt
        emit_pair_fin(NH // 2 - 1, prev)

    if not have_attn:
        for sc in range(SC):
            nc.vector.tensor_scalar(out=outA[:, sc, :], in0=ones_f32[:, :],
                                    scalar1=0.1, scalar2=None, op0=OP.mult)
    if not have_attn:
        nc.vector.tensor_copy(sq[:, :, :], outA[:, :, :])
    # ---- tail: BN stats + AllReduce, scale/shift, output ----
    # (residual + square already happened per pair inside emit_pair_fin)
    sum_ps = psum.tile([1, 512], F32, tag="score_ps", name="sum_ps", bufs=2)
    sq_ps = psum.tile([1, 512], F32, tag="u_ps", name="sq_ps")
    for sc in range(SC):
        nc.tensor.matmul(sum_ps[:, :], ones_col_r, outA[:, sc, :],
                         start=(sc == 0), stop=(sc == SC - 1))
        nc.tensor.matmul(sq_ps[:, :], ones_col_r, sq[:, sc, :],
                         start=(sc == 0), stop=(sc == SC - 1))

    stats_sb = statp.tile([1, 1024], F32, tag="stats_sb", name="stats_sb")
    nc.vector.tensor_copy(stats_sb[:, 0:512], sum_ps[:, :])
    nc.vector.tensor_copy(stats_sb[:, 512:1024], sq_ps[:, :])
    if with_tail:
        dram = ctx.enter_context(tc.tile_pool(name="dram", bufs=1, space="DRAM"))
        cc_in = dram.tile([1, 1024], F32)
        cc_out = dram.tile([1, 1024], F32)
        nc.sync.dma_start(cc_in[:, :], stats_sb[:, :])
        nc.gpsimd.collective_compute(
            "AllReduce", OP.add,
            replica_groups=[list(range(N_CORES))],
            ins=[cc_in[:, :].opt()], outs=[cc_out[:, :].opt()])
        gstats = statp.tile([1, 1024], F32, tag="gstats", name="gstats")
        nc.sync.dma_start(gstats[:, :], cc_out[:, :])
    else:
        # timing-only build: skip collective (banned in control flow) but run
        # the full tail math on local stats so engine load is representative
        gstats = stats_sb
    mean = statp.tile([1, 512], F32, tag="mean", name="mean")
    nc.vector.tensor_scalar(out=mean[:, :], in0=gstats[:, 0:512],
                            scalar1=1.0 / N_ROWS_TOTAL, scalar2=None,
                            op0=OP.mult)
    esq = statp.tile([1, 512], F32, tag="esq", name="esq")
    nc.vector.tensor_scalar(out=esq[:, :], in0=gstats[:, 512:1024],
                            scalar1=1.0 / N_ROWS_TOTAL, scalar2=None,
                            op0=OP.mult)
    var = statp.tile([1, 512], F32, tag="var", name="var")
    nc.vector.tensor_mul(var[:, :], mean[:, :], mean[:, :])
    nc.vector.tensor_sub(var[:, :], esq[:, :], var[:, :])
    # ve = var + eps; y = rsqrt(ve) via Quake bit-trick + 1 Newton step
    # (keeps the whole tail off ACT so the exp table set never swaps)
    ve = statp.tile([1, 512], F32, tag="ve", name="ve")
    nc.vector.tensor_scalar(out=ve[:, :], in0=var[:, :], scalar1=BN_EPS,
                            scalar2=None, op0=OP.add)
    ybits = statp.tile([1, 512], I32, tag="ybits", name="ybits")
    nc.vector.tensor_scalar(out=ybits[:, :], in0=ve[:, :].bitcast(I32),
                            scalar1=1, scalar2=None,
                            op0=OP.arith_shift_right)
    nc.vector.tensor_scalar(out=ybits[:, :], in0=ybits[:, :],
                            scalar1=-1, scalar2=None, op0=OP.bitwise_xor)
    y = statp.tile([1, 512], F32, tag="y", name="y")
    nc.vector.tensor_scalar(out=y[:, :].bitcast(I32), in0=ybits[:, :],
                            scalar1=QUAKE_C + 1, scalar2=None, op0=OP.add)
    t = statp.tile([1, 512], F32, tag="t", name="t")
    for _ in range(1):
        nc.vector.tensor_mul(t[:, :], y[:, :], y[:, :])
        nc.vector.tensor_mul(t[:, :], t[:, :], ve[:, :])
        nc.vector.tensor_scalar(out=t[:, :], in0=t[:, :], scalar1=-0.5,
                                scalar2=1.5, op0=OP.mult, op1=OP.add)
        nc.vector.tensor_mul(y[:, :], y[:, :], t[:, :])
    A = statp.tile([1, 512], F32R, tag="A", name="A")
    nc.vector.tensor_mul(A[:, :], y[:, :], gamma_sb[:, :])
    B = statp.tile([1, 512], F32R, tag="Bt", name="Bt")
    nc.vector.tensor_mul(B[:, :], mean[:, :], A[:, :])
    nc.vector.tensor_sub(B[:, :], beta_sb[:, :], B[:, :])
    a_ps = psum.tile([128, 512], F32, tag="score_ps", name="a_ps", bufs=2)
    b_ps = psum.tile([128, 512], F32, tag="u_ps", name="b_ps")
    nc.tensor.matmul(a_ps[:, :], ones_row_r, A[:, :],
                     start=True, stop=True)
    nc.tensor.matmul(b_ps[:, :], ones_row_r, B[:, :],
                     start=True, stop=True)
    t2 = tmpp.tile([128, SC, 512], F32, tag="t2", name="t2", bufs=1)
    outr = out[:, :].rearrange("(r p) c -> p r c", p=128)
    if T2_GPS:
        # split scale/shift between GPSIMD (needs SBUF-staged A/B) and DVE
        # (reads the PSUM broadcast directly) to halve the serial tail
        a_sb = statp.tile([128, 512], F32, tag="a_sb", name="a_sb")
        b_sb = statp.tile([128, 512], F32, tag="b_sb", name="b_sb")
        nc.vector.tensor_copy(a_sb[:, :], a_ps[:, :])
        nc.vector.tensor_copy(b_sb[:, :], b_ps[:, :])
        for g in range(4):
            lo, hi = g * 2, (g + 1) * 2
            r0, r1 = lo, lo + 1  # r0 -> GPS, r1 -> DVE, concurrent
            nc.gpsimd.tensor_mul(t2[:, r0, :], outA[:, r0, :], a_sb[:, :])
            in0, in1 = bass.broadcast_tensor_aps(outA[:, r1:r1 + 1, :],
                                                 a_ps[:, :].unsqueeze(1))
            nc.vector.tensor_mul(t2[:, r1:r1 + 1, :], in0, in1)
            nc.gpsimd.tensor_add(t2[:, r0, :], t2[:, r0, :], b_sb[:, :])
            in0, in1 = bass.broadcast_tensor_aps(t2[:, r1:r1 + 1, :],
                                                 b_ps[:, :].unsqueeze(1))
            nc.vector.tensor_add(t2[:, r1:r1 + 1, :], in0, in1)
            nc.sync.dma_start(outr[:, lo:hi, :], t2[:, lo:hi, :])
    else:
        hh = SC // 2
        for h0, h1 in ((0, hh), (hh, SC)):
            in0, in1 = bass.broadcast_tensor_aps(outA[:, h0:h1, :],
                                                 a_ps[:, :].unsqueeze(1))
            nc.vector.tensor_mul(t2[:, h0:h1, :], in0, in1)
            in0, in1 = bass.broadcast_tensor_aps(t2[:, h0:h1, :],
                                                 b_ps[:, :].unsqueeze(1))
            nc.vector.tensor_add(t2[:, h0:h1, :], in0, in1)
            nc.sync.dma_start(outr[:, h0:h1, :], t2[:, h0:h1, :])
    ctx.close()


def build_nc(reps=1, parts=("prep", "attn"), with_tail=None):
    nc = bacc.Bacc("TRN2", target_bir_lowering=False, debug=False)
    x = nc.dram_tensor("x", [S, H], F32, kind="ExternalInput")
    wq = nc.dram_tensor("wq", [H, H], F32, kind="ExternalInput")
    bq = nc.dram_tensor("bq", [H], F32, kind="ExternalInput")
    wk = nc.dram_tensor("wk", [H, H], F32, kind="ExternalInput")
    bk = nc.dram_tensor("bk", [H], F32, kind="ExternalInput")
    wv = nc.dram_tensor("wv", [H, H], F32, kind="ExternalInput")
    bv = nc.dram_tensor("bv", [H], F32, kind="ExternalInput")
    gamma = nc.dram_tensor("gamma", [H], F32, kind="ExternalInput")
    beta = nc.dram_tensor("beta", [H], F32, kind="ExternalInput")
    out = nc.dram_tensor("out", [S, H], F32, kind="ExternalOutput")
    tens = (x, wq, bq, wk, bk, wv, bv, gamma, beta, out)

    with ExitStack() as ctx:
        tc = ctx.enter_context(tile.TileContext(nc))
        if with_tail is None:
            with_tail = (reps == 1)
        if reps == 1:
            emit_body(nc, tc, ctx, tens, with_tail=with_tail, parts=parts)
        else:
            hints = (mybir.EngineType.PE, mybir.EngineType.DVE,
                     mybir.EngineType.Activation, mybir.EngineType.SP)
            with tc.For_i(0, reps, 1, hint_engines=hints):
                emit_body(nc, tc, ctx, tens, with_tail=False, parts=parts)
    nc.compile()
    return nc


_CACHED_NC = None


def kernel(**inputs):
    global _CACHED_NC
    x_full = np.ascontiguousarray(np.asarray(inputs["inputs"], dtype=np.float32))
    args = {k: np.ascontiguousarray(np.asarray(inputs[k], dtype=np.float32))
            for k in ("wq", "bq", "wk", "bk", "wv", "bv", "gamma", "beta")}
    if _CACHED_NC is None:
        _CACHED_NC = build_nc(reps=1)
    nc = _CACHED_NC
    in_maps = []
    for b in range(N_CORES):
        m = {"x": x_full[b]}
        m.update(args)
        in_maps.append(m)
    res = run_bass_kernel_spmd(nc, in_maps, list(range(N_CORES)))
    out = np.stack([res.results[b]["out"] for b in range(N_CORES)], axis=0)
    return out.astype(np.float32)


# revision 9
# speedup vs baseline: 1.1454x; 1.1454x over previous
"""Trainium2 Bass kernel for nn_MultiHeadAttention_23467701305746.

Reference computation (batch 8, seq 1024, hidden 512, 16 heads x 32):
  q/k/v = relu(x @ W + b); scores = q k^T / sqrt(32); attn = softmax(scores)
  out = attn @ v + x;  BatchNorm1d over (batch, seq) per channel, eps=1e-3.

Sharding: data-parallel over batch, 1 batch element per NeuronCore (8 cores).
BatchNorm batch statistics are combined with a tiny (4 KB) AllReduce.

The kernel is softmax-exp bound: 16.8M exp elements/core = 131072/lane.
ACT runs exp at 1 elem/cycle/lane (1.2 GHz), so exp alone would be ~110us.
v2 splits exp across TWO engines working on alternating score chunks:
  - ACT: true exp (PSUM f32 -> SBUF bf16), (FD+222)/1.2 ns per chunk.
  - DVE: one-op Schraudolph: bf16_bits = round_i16(s * A7 + B7) with
    A7 = log2(e)/sqrt(32)*128, B7 ~ 16249.5. The f32->i16 output convert
    rounds to nearest (hardware-verified), so a single tensor_scalar
    (mult, add) from PSUM produces the bf16 exp approximation directly.
    Elementwise error +-3.3%; softmax ratio cancellation keeps the final
    output error ~3e-4 (verified in numpy bit-exact sim).
Remaining elementwise work balances: relu epilogues / U evacuations split
ACT/DVE by knobs; residual add, square (BN variance), and the final BN
scale/shift run on GPSIMD; weight f32->bf16 casts ride on gpsimd SWDGE
DMA (descriptor cast). Per head pair (2 heads row/col-packed per PE pass):
  S^T[k,q] = kT_h^T qT_h (row-tiled x2); exp chunk [128,2,512] -> bf16
  One 33-col matmul per head gives U^T rows AND the softmax denominator
  row in a single pass (stationary = [v_h | 1]).
U^T chunks --PE bf16 transpose--> layout A; strided reciprocal + broadcast
multiply normalizes; residual add + BN partial sums (ones-matmul)
-> AllReduce -> scale/shift (Quake rsqrt on DVE; no ACT table swap) -> out.
"""

import math
import numpy as np
from contextlib import ExitStack

import concourse.bass as bass
import concourse.tile as tile
from concourse import bacc, mybir
from concourse.bass_utils import run_bass_kernel_spmd
from concourse.masks import make_identity

F32 = mybir.dt.float32
F32R = mybir.dt.float32r
BF16 = mybir.dt.bfloat16
I32 = mybir.dt.int32
I16 = mybir.dt.int16
OP = mybir.AluOpType
AF = mybir.ActivationFunctionType

N_CORES = 8
S = 1024          # sequence length per core (= per batch element)
H = 512           # hidden
NH = 16           # heads
D = 32            # head dim
KC = H // 128     # 4 contraction chunks over hidden
SC = S // 128     # 8 chunks over sequence
QH = S // 512     # 2 query halves (N=512 fp32 matmul limit)
BN_EPS = 1e-3
INV_SQRT_D = 1.0 / math.sqrt(D)
N_ROWS_TOTAL = 8 * S  # BN stats denominator (batch*seq)
QUAKE_C = 0x5F3759DF

# Schraudolph exp constants (bf16 bit pattern = round(s*A7 + B7))
EXP_A7 = float(math.log2(math.e) / math.sqrt(D) * 128.0)
EXP_B7 = 16249.5

# ---- balance knobs ----
EXP_ACT_NUM = 62      # score chunks of 128 assigned to ACT (rest DVE)
XT_EVAC_ACT = 4       # of the 8 xT evac chunks, how many on ACT (rest DVE)
US_EVAC_ACT = 8       # of the 8 U evacuations, how many on ACT (rest DVE)
V_RELU_DVE = False    # v-projection relu epilogue on DVE instead of ACT
RESID_GPS = True      # residual add on GPSIMD
SQ_GPS = True         # BN square on GPSIMD
T2_GPS = True         # BN scale/shift on GPSIMD
CONVW_GPS_DMA = True  # weight f32->bf16 converts via gpsimd dma cast


def _exp_engine(chunk_idx):
    """Deterministic ACT/DVE interleave with EXP_ACT_NUM/128 chunks on ACT,
    spread evenly so the engines alternate rather than phase-separate."""
    pos = chunk_idx % 128
    acc_before = (pos * EXP_ACT_NUM) // 128
    acc_after = ((pos + 1) * EXP_ACT_NUM) // 128
    return "act" if acc_after > acc_before else "dve"


def emit_body(nc, tc, outer_ctx, tens, with_tail=True, parts=("prep", "attn")):
    x, wq, bq, wk, bk, wv, bv, gamma, beta, out = tens
    ctx = outer_ctx.enter_context(ExitStack())

    const = ctx.enter_context(tc.tile_pool(name="const", bufs=1))
    xpool = ctx.enter_context(tc.tile_pool(name="xpool", bufs=1))
    qkp = ctx.enter_context(tc.tile_pool(name="qkp", bufs=1))
    vpool = ctx.enter_context(tc.tile_pool(name="vpool", bufs=1))
    statp = ctx.enter_context(tc.tile_pool(name="statp", bufs=1))
    psum = ctx.enter_context(tc.tile_pool(name="psum", bufs=1, space="PSUM"))

    # ---- constants ----
    ident = const.tile([128, 128], F32, tag="ident", name="ident")
    make_identity(nc, ident[:, :])
    ident_bf = const.tile([128, 128], BF16, tag="ident_bf", name="ident_bf")
    nc.vector.tensor_copy(ident_bf[:, :], ident[:, :])
    ones_f32 = const.tile([128, 512], F32, tag="ones_f32", name="ones_f32")
    nc.vector.memset(ones_f32[:, :], 1.0)
    exp_warm = const.tile([1, 1], F32, tag="exp_warm", name="exp_warm")
    nc.scalar.activation(exp_warm[:, :], ones_f32[0:1, 0:1], AF.Exp)
    warm_mv = const.tile([128, 512], BF16, tag="warm_mv", name="warm_mv")
    nc.vector.memset(warm_mv[:, :], 0.0)
    wps = psum.tile([128, 512], F32, tag="score_ps", name="tps", bufs=3)
    for _ in range(10):
        nc.tensor.matmul(wps[:, :], ident_bf[:, :], warm_mv[:, :],
                         start=True, stop=True)
    ones_row_b = const.tile([1, 128], BF16, tag="ones_row_b", name="ones_row_b")  # K=1 lhsT
    nc.vector.tensor_copy(ones_row_b[:, :], ones_f32[0:1, 0:128])
    ones_row_r = const.tile([1, 128], F32R, tag="ones_row_r", name="ones_row_r")
    nc.vector.tensor_copy(ones_row_r[:, :], ones_f32[0:1, 0:128])
    ones_col_r = const.tile([128, 1], F32R, tag="ones_col_r", name="ones_col_r")
    nc.vector.tensor_copy(ones_col_r[:, :], ones_f32[:, 0:1])
    gamma_sb = const.tile([1, 512], F32, tag="gamma", name="gamma")
    beta_sb = const.tile([1, 512], F32, tag="beta", name="beta")

    # ---- x load (strided DMAs so transposes start early) ----
    x_sb = xpool.tile([128, SC, 512], F32, tag="x_sb", name="x_sb")
    xr = x[:, :].rearrange("(r p) c -> p r c", p=128)
    for q4 in range(4):
        lo, hi = q4 * (SC // 4), (q4 + 1) * (SC // 4)
        nc.sync.dma_start(x_sb[:, lo:hi, :], xr[:, lo:hi, :])

    qT = [qkp.tile([128, S], BF16, tag=f"qT{c}", name=f"qT{c}") for c in range(KC)]
    kT = [qkp.tile([128, S], BF16, tag=f"kT{c}", name=f"kT{c}") for c in range(KC)]
    # v is stored as [128k, head, 33]: columns 0:32 are v_h, column 32 is 1.0
    # so one U matmul per head yields U rows plus the softmax denominator row
    v_r = [vpool.tile([128, NH, 33], BF16, tag=f"v{kv}", name=f"v{kv}") for kv in range(SC)]

    if "prep" not in parts:
        nc.sync.dma_start(gamma_sb[:, :], gamma[:].unsqueeze(0))
        nc.sync.dma_start(beta_sb[:, :], beta[:].unsqueeze(0))
        # bisection mode: fill qT/kT/v with memset instead of real projections
        for c in range(KC):
            nc.vector.memset(qT[c][:, :], 0.01)
            nc.vector.memset(kT[c][:, :], 0.01)
        for kv in range(SC):
            nc.vector.memset(v_r[kv][:, :, :], 0.5)
            nc.vector.tensor_copy(v_r[kv][:, :, 32:33], ones_f32[:, 0:NH].unsqueeze(2))
    # ---- pools (prep + attention live together for the interleave) ----
    expp = ctx.enter_context(tc.tile_pool(name="expp", bufs=2))
    usb = ctx.enter_context(tc.tile_pool(name="usb", bufs=2))
    outp = ctx.enter_context(tc.tile_pool(name="outp", bufs=1))
    tmpp = ctx.enter_context(tc.tile_pool(name="tmpp", bufs=2))
    outA = outp.tile([128, SC, 512], F32R, tag="outA", name="outA")

    have_prep = "prep" in parts
    have_attn = "attn" in parts
    exp_chunk_counter = [0]

    if have_prep:
        wpool = ctx.enter_context(tc.tile_pool(name="wpool", bufs=1))
        wstg = ctx.enter_context(tc.tile_pool(name="wstg", bufs=1))
        xtp = ctx.enter_context(tc.tile_pool(name="xtp", bufs=1))

        # one strided DMA per weight matrix into f32 staging [128p, kc, 512c]
        # (p = contraction sub-index), then one cast to bf16
        w_st, w_r = {}, {}
        for name, wt in (("q", wq), ("k", wk), ("v", wv)):
            st = wstg.tile([128, KC, 512], F32, tag=f"ws{name}", name=f"ws{name}")
            nc.sync.dma_start(st[:, :, :],
                              wt[:, :].rearrange("(k p) c -> p k c", p=128))
            w_st[name] = st
            w_r[name] = wpool.tile([128, KC, 512], BF16, tag=f"w{name}",
                                   name=f"w{name}")
        bqk = wpool.tile([128, 2, KC], F32, tag="bqk", name="bqk")
        nc.sync.dma_start(bqk[:, 0, :], bq[:].rearrange("(k p) -> p k", p=128))
        nc.sync.dma_start(bqk[:, 1, :], bk[:].rearrange("(k p) -> p k", p=128))
        bv_sb = wpool.tile([1, 512], F32, tag="bv_sb", name="bv_sb")
        nc.sync.dma_start(bv_sb[:, :], bv[:].unsqueeze(0))
        nc.sync.dma_start(gamma_sb[:, :], gamma[:].unsqueeze(0))
        nc.sync.dma_start(beta_sb[:, :], beta[:].unsqueeze(0))
        bv_b = wpool.tile([1, 512], BF16, tag="bv_b", name="bv_b")
        nc.vector.tensor_copy(bv_b[:, :], bv_sb[:, :])

        def conv_w(name):
            if CONVW_GPS_DMA:
                nc.gpsimd.dma_start(
                    w_r[name][:, :, :].rearrange("p a b -> p (a b)"),
                    w_st[name][:, :, :].rearrange("p a b -> p (a b)"))
            else:
                nc.vector.tensor_copy(
                    w_r[name][:, :, :].rearrange("p a b -> p (a b)"),
                    w_st[name][:, :, :].rearrange("p a b -> p (a b)"))

        # q/k weight converts first (they gate the first projections)
        conv_w("q")
        conv_w("k")

        # x transpose -> xT (bf16); PSUM evacuation split ACT/DVE
        xT = [xtp.tile([128, S], BF16, tag=f"xT{c}", name=f"xT{c}") for c in range(KC)]
        for c in range(KC):
            for half in range(2):
                tp4 = psum.tile([128, 4, 128], F32, tag="score_ps",
                                name="tps", bufs=3)
                for rr in range(4):
                    r = half * 4 + rr
                    nc.tensor.transpose(tp4[:, rr, :],
                                        x_sb[:, r, c * 128:(c + 1) * 128],
                                        ident[:, :])
                dst = xT[c][:, half * 512:(half + 1) * 512]
                src = tp4[:, :, :].rearrange("p a b -> p (a b)")
                if c * 2 + half < XT_EVAC_ACT:
                    nc.scalar.copy(dst, src)
                else:
                    nc.vector.tensor_copy(dst, src)

        def proj_qk(oc):
            # projections q,k chunk oc (layout B); relu+bias fused on ACT
            for wi, (wkey, dest) in enumerate((("q", qT), ("k", kT))):
                for sh in range(QH):
                    ps = psum.tile([128, 512], F32, tag="score_ps",
                                   name="tps", bufs=3)
                    for kc in range(KC):
                        nc.tensor.matmul(
                            ps[:, :],
                            w_r[wkey][:, kc, oc * 128:(oc + 1) * 128],
                            xT[kc][:, sh * 512:(sh + 1) * 512],
                            start=(kc == 0), stop=(kc == KC - 1))
                    nc.scalar.activation(
                        dest[oc][:, sh * 512:(sh + 1) * 512],
                        ps[:, :], AF.Relu, bias=bqk[:, wi, oc:oc + 1],
                        scale=1.0)

        def proj_v():
            # v (layout A); bias via ones-matmul, relu on ACT or DVE
            conv_w("v")
            for kv in range(SC):
                ps = psum.tile([128, 512], F32, tag="score_ps",
                               name="tps", bufs=3)
                for kc in range(KC):
                    nc.tensor.matmul(ps[:, :], xT[kc][:, kv * 128:(kv + 1) * 128],
                                     w_r["v"][:, kc, :],
                                     start=(kc == 0), stop=False)
                nc.tensor.matmul(ps[:, :], ones_row_b[:, :], bv_b[:, :],
                                 start=False, stop=True)
                dst = v_r[kv][:, :, 0:32]
                src = ps[:, :].rearrange("p (h d) -> p h d", h=NH)
                if V_RELU_DVE:
                    nc.vector.tensor_scalar(out=dst, in0=src, scalar1=0.0,
                                            scalar2=None, op0=OP.max)
                else:
                    nc.scalar.activation(dst, src, AF.Relu)
                nc.vector.tensor_copy(v_r[kv][:, :, 32:33], ones_f32[:, 0:NH].unsqueeze(2))

    def emit_pair_mm(hp):
        h0, h1 = 2 * hp, 2 * hp + 1
        ch = h0 // 4                 # qT/kT chunk holding these heads
        p0 = (h0 % 4) * 32           # partition base of h0 within chunk
        p1 = (h1 % 4) * 32
        ups = psum.tile([128, S], F32, tag="u_ps", name="u_ps")

        def emit_scores(kc, qh):
            # half-size chunk with a double-buffered PSUM tag so the PE can
            # run one chunk ahead of the exp engines instead of stalling
            sps = psum.tile([128, 2, 512], F32, tag="score_ps",
                            name="score_ps", bufs=3)
            for j, pb in ((0, p0), (1, p1)):
                nc.tensor.matmul(
                    sps[:, j, :],
                    kT[ch][pb:pb + 32, kc * 128:(kc + 1) * 128],
                    qT[ch][pb:pb + 32, qh * 512:(qh + 1) * 512],
                    start=True, stop=True, tile_position=(pb, 0))
            ex = expp.tile([128, 2, 512], BF16, tag="expS", name="expS",
                           bufs=4)
            # alternate exp between ACT (true exp) and DVE (Schraudolph)
            if _exp_engine(exp_chunk_counter[0]) == "act":
                nc.scalar.activation(
                    ex[:, :, :].rearrange("p a c -> p (a c)"),
                    sps[:, :, :].rearrange("p a c -> p (a c)"),
                    AF.Exp, scale=INV_SQRT_D)
            else:
                nc.vector.tensor_scalar(
                    out=ex[:, :, :].bitcast(I16), in0=sps[:, :, :],
                    scalar1=EXP_A7, scalar2=EXP_B7,
                    op0=OP.mult, op1=OP.add)
            exp_chunk_counter[0] += 1
            return ex

        def emit_u(kc, qh, ex):
            # one 33-col matmul per head: rows 0:32 = U_h, row 32 = rowsum
            st, sp = (kc == 0), (kc == SC - 1)
            q0, q1 = qh * 512, (qh + 1) * 512
            nc.tensor.matmul(ups[0:33, q0:q1], v_r[kc][:, h0, :],
                             ex[:, 0, :], start=st, stop=sp,
                             tile_position=(0, 0))
            nc.tensor.matmul(ups[64:97, q0:q1], v_r[kc][:, h1, :],
                             ex[:, 1, :], start=st, stop=sp,
                             tile_position=(0, 64))

        chunks = [(kc, qh) for kc in range(SC) for qh in range(QH)]
        prev_c, prev_ex = chunks[0], emit_scores(*chunks[0])
        for cur in chunks[1:]:
            cur_ex = emit_scores(*cur)
            emit_u(prev_c[0], prev_c[1], prev_ex)
            prev_c, prev_ex = cur, cur_ex
        emit_u(prev_c[0], prev_c[1], prev_ex)

        # evacuate U (rows [0:32]=U_h0, [32]=rowsum_h0, [64:96]=U_h1,
        # [96]=rowsum_h1) to bf16 SBUF; transposes happen in _fin
        us = usb.tile([128, S], BF16, tag="u_sb", name="u_sb")
        if hp < US_EVAC_ACT:
            nc.scalar.copy(us[:, :], ups[:, :])
        else:
            nc.vector.tensor_copy(us[:, :], ups[:, :])
        return us

    sq = tmpp.tile([128, SC, 512], F32R, tag="sq", name="sq", bufs=1)

    def emit_pair_fin(hp, us):
        # transpose each seq chunk to layout A and divide by rowsum column;
        # all 8 bf16 transposes of the pair pack into one PSUM bank tile
        tpb = psum.tile([128, SC, 128], BF16, tag="score_ps", name="tps",
                        bufs=3)
        for sc in range(SC):
            nc.tensor.transpose(tpb[:, sc, :], us[:, sc * 128:(sc + 1) * 128],
                                ident_bf[:, :])
        rsr = statp.tile([128, SC, 2], F32, tag="rsr", name="rsr", bufs=2)
        # one strided reciprocal for both rowsum columns, one fused
        # broadcast-multiply for both heads (4D strided views)
        nc.vector.reciprocal(rsr[:, :, 0:2], tpb[:, :, 32:97:64])
        in0 = tpb[:, :, :].rearrange("p s (b c) -> p s b c", b=4)[:, :, 0:3:2, :]
        ov = outA[:, :, 64 * hp:64 * (hp + 1)].rearrange(
            "p s (b c) -> p s b c", b=2)
        in0b, in1b = bass.broadcast_tensor_aps(in0, rsr[:, :, 0:2].unsqueeze(3))
        nc.vector.tensor_mul(ov, in0b, in1b)
        # residual + BN square for this pair's 64 output columns, on GPSIMD
        # (otherwise idle) so the serial tail after the last pair shrinks
        c0, c1 = 64 * hp, 64 * (hp + 1)
        if RESID_GPS:
            nc.gpsimd.tensor_add(outA[:, :, c0:c1], outA[:, :, c0:c1],
                                 x_sb[:, :, c0:c1])
        else:
            nc.vector.tensor_add(outA[:, :, c0:c1], outA[:, :, c0:c1],
                                 x_sb[:, :, c0:c1])
        if SQ_GPS:
            nc.gpsimd.tensor_mul(sq[:, :, c0:c1], outA[:, :, c0:c1],
                                 outA[:, :, c0:c1])
        else:
            nc.vector.tensor_mul(sq[:, :, c0:c1], outA[:, :, c0:c1],
                                 outA[:, :, c0:c1])

    # interleaved emission: projections feed attention pairs chunk by chunk
    # so PE projection work hides under exp work (per-engine queues are
    # in-order; emission order controls overlap). Each pair's evac
    # transposes are deferred past the next projection chunk so the PE
    # doesn't idle waiting on the us-copy.
    if have_prep and have_attn:
        proj_qk(0)
        proj_v()
        usd = {}
        usd[0] = emit_pair_mm(0)
        proj_qk(1)
        usd[1] = emit_pair_mm(1)
        emit_pair_fin(0, usd[0])
        proj_qk(2)
        usd[2] = emit_pair_mm(2)
        emit_pair_fin(1, usd[1])
        proj_qk(3)
        usd[3] = emit_pair_mm(3)
        emit_pair_fin(2, usd[2])
        for hp in range(4, NH // 2):
            usd[hp] = emit_pair_mm(hp)
            emit_pair_fin(hp - 1, usd[hp - 1])
        emit_pair_fin(NH // 2 - 1, usd[NH // 2 - 1])
    elif have_prep:
        for oc in range(KC):
            proj_qk(oc)
        proj_v()
    elif have_attn:
        prev = emit_pair_mm(0)
        for hp in range(1, NH // 2):
            nxt = emit_pair_mm(hp)
            emit_pair_fin(hp - 1, prev)
            prev = nxt
        emit_pair_fin(NH // 2 - 1, prev)

    if not have_attn:
        for sc in range(SC):
            nc.vector.tensor_scalar(out=outA[:, sc, :], in0=ones_f32[:, :],
                                    scalar1=0.1, scalar2=None, op0=OP.mult)
    if not have_attn:
        nc.vector.tensor_copy(sq[:, :, :], outA[:, :, :])
    # ---- tail: BN stats + AllReduce, scale/shift, output ----
    # (residual + square already happened per pair inside emit_pair_fin)
    sum_ps = psum.tile([1, 512], F32, tag="score_ps", name="sum_ps", bufs=3)
    sq_ps = psum.tile([1, 512], F32, tag="u_ps", name="sq_ps")
    for sc in range(SC):
        nc.tensor.matmul(sum_ps[:, :], ones_col_r, outA[:, sc, :],
                         start=(sc == 0), stop=(sc == SC - 1))
        nc.tensor.matmul(sq_ps[:, :], ones_col_r, sq[:, sc, :],
                         start=(sc == 0), stop=(sc == SC - 1))

    stats_sb = statp.tile([1, 1024], F32, tag="stats_sb", name="stats_sb")
    nc.vector.tensor_copy(stats_sb[:, 0:512], sum_ps[:, :])
    nc.vector.tensor_copy(stats_sb[:, 512:1024], sq_ps[:, :])
    if with_tail:
        dram = ctx.enter_context(tc.tile_pool(name="dram", bufs=1, space="DRAM"))
        cc_in = dram.tile([1, 1024], F32)
        cc_out = dram.tile([1, 1024], F32)
        nc.sync.dma_start(cc_in[:, :], stats_sb[:, :])
        nc.gpsimd.collective_compute(
            "AllReduce", OP.add,
            replica_groups=[list(range(N_CORES))],
            ins=[cc_in[:, :].opt()], outs=[cc_out[:, :].opt()])
        gstats = statp.tile([1, 1024], F32, tag="gstats", name="gstats")
        nc.sync.dma_start(gstats[:, :], cc_out[:, :])
    else:
        # timing-only build: skip collective (banned in control flow) but run
        # the full tail math on local stats so engine load is representative
        gstats = stats_sb
    mean = statp.tile([1, 512], F32, tag="mean", name="mean")
    nc.vector.tensor_scalar(out=mean[:, :], in0=gstats[:, 0:512],
                            scalar1=1.0 / N_ROWS_TOTAL, scalar2=None,
                            op0=OP.mult)
    esq = statp.tile([1, 512], F32, tag="esq", name="esq")
    nc.vector.tensor_scalar(out=esq[:, :], in0=gstats[:, 512:1024],
                            scalar1=1.0 / N_ROWS_TOTAL, scalar2=None,
                            op0=OP.mult)
    var = statp.tile([1, 512], F32, tag="var", name="var")
    nc.vector.tensor_mul(var[:, :], mean[:, :], mean[:, :])
    nc.vector.tensor_sub(var[:, :], esq[:, :], var[:, :])
    # ve = var + eps; y = rsqrt(ve) via Quake bit-trick + 1 Newton step
    # (keeps the whole tail off ACT so the exp table set never swaps)
    ve = statp.tile([1, 512], F32, tag="ve", name="ve")
    nc.vector.tensor_scalar(out=ve[:, :], in0=var[:, :], scalar1=BN_EPS,
                            scalar2=None, op0=OP.add)
    ybits = statp.tile([1, 512], I32, tag="ybits", name="ybits")
    nc.vector.tensor_scalar(out=ybits[:, :], in0=ve[:, :].bitcast(I32),
                            scalar1=1, scalar2=None,
                            op0=OP.arith_shift_right)
    nc.vector.tensor_scalar(out=ybits[:, :], in0=ybits[:, :],
                            scalar1=-1, scalar2=None, op0=OP.bitwise_xor)
    y = statp.tile([1, 512], F32, tag="y", name="y")
    nc.vector.tensor_scalar(out=y[:, :].bitcast(I32), in0=ybits[:, :],
                            scalar1=QUAKE_C + 1, scalar2=None, op0=OP.add)
    t = statp.tile([1, 512], F32, tag="t", name="t")
    for _ in range(1):
        nc.vector.tensor_mul(t[:, :], y[:, :], y[:, :])
        nc.vector.tensor_mul(t[:, :], t[:, :], ve[:, :])
        nc.vector.tensor_scalar(out=t[:, :], in0=t[:, :], scalar1=-0.5,
                                scalar2=1.5, op0=OP.mult, op1=OP.add)
        nc.vector.tensor_mul(y[:, :], y[:, :], t[:, :])
    A = statp.tile([1, 512], F32R, tag="A", name="A")
    nc.vector.tensor_mul(A[:, :], y[:, :], gamma_sb[:, :])
    B = statp.tile([1, 512], F32R, tag="Bt", name="Bt")
    nc.vector.tensor_mul(B[:, :], mean[:, :], A[:, :])
    nc.vector.tensor_sub(B[:, :], beta_sb[:, :], B[:, :])
    a_ps = psum.tile([128, 512], F32, tag="score_ps", name="a_ps", bufs=3)
    b_ps = psum.tile([128, 512], F32, tag="u_ps", name="b_ps")
    nc.tensor.matmul(a_ps[:, :], ones_row_r, A[:, :],
                     start=True, stop=True)
    nc.tensor.matmul(b_ps[:, :], ones_row_r, B[:, :],
                     start=True, stop=True)
    t2 = tmpp.tile([128, SC, 512], F32, tag="t2", name="t2", bufs=1)
    outr = out[:, :].rearrange("(r p) c -> p r c", p=128)
    if T2_GPS:
        # split scale/shift between GPSIMD (needs SBUF-staged A/B) and DVE
        # (reads the PSUM broadcast directly) to halve the serial tail
        a_sb = statp.tile([128, 512], F32, tag="a_sb", name="a_sb")
        b_sb = statp.tile([128, 512], F32, tag="b_sb", name="b_sb")
        nc.vector.tensor_copy(a_sb[:, :], a_ps[:, :])
        nc.vector.tensor_copy(b_sb[:, :], b_ps[:, :])
        for g in range(4):
            lo, hi = g * 2, (g + 1) * 2
            r0, r1 = lo, lo + 1  # r0 -> GPS, r1 -> DVE, concurrent
            nc.gpsimd.tensor_mul(t2[:, r0, :], outA[:, r0, :], a_sb[:, :])
            in0, in1 = bass.broadcast_tensor_aps(outA[:, r1:r1 + 1, :],
                                                 a_ps[:, :].unsqueeze(1))
            nc.vector.tensor_mul(t2[:, r1:r1 + 1, :], in0, in1)
            nc.gpsimd.tensor_add(t2[:, r0, :], t2[:, r0, :], b_sb[:, :])
            in0, in1 = bass.broadcast_tensor_aps(t2[:, r1:r1 + 1, :],
                                                 b_ps[:, :].unsqueeze(1))
            nc.vector.tensor_add(t2[:, r1:r1 + 1, :], in0, in1)
            nc.sync.dma_start(outr[:, lo:hi, :], t2[:, lo:hi, :])
    else:
        hh = SC // 2
        for h0, h1 in ((0, hh), (hh, SC)):
            in0, in1 = bass.broadcast_tensor_aps(outA[:, h0:h1, :],
                                                 a_ps[:, :].unsqueeze(1))
            nc.vector.tensor_mul(t2[:, h0:h1, :], in0, in1)
            in0, in1 = bass.broadcast_tensor_aps(t2[:, h0:h1, :],
                                                 b_ps[:, :].unsqueeze(1))
            nc.vector.tensor_add(t2[:, h0:h1, :], in0, in1)
            nc.sync.dma_start(outr[:, h0:h1, :], t2[:, h0:h1, :])
    ctx.close()


def build_nc(reps=1, parts=("prep", "attn"), with_tail=None):
    nc = bacc.Bacc("TRN2", target_bir_lowering=False, debug=False)
    x = nc.dram_tensor("x", [S, H], F32, kind="ExternalInput")
    wq = nc.dram_tensor("wq", [H, H], F32, kind="ExternalInput")
    bq = nc.dram_tensor("bq", [H], F32, kind="ExternalInput")
    wk = nc.dram_tensor("wk", [H, H], F32, kind="ExternalInput")
    bk = nc.dram_tensor("bk", [H], F32, kind="ExternalInput")
    wv = nc.dram_tensor("wv", [H, H], F32, kind="ExternalInput")
    bv = nc.dram_tensor("bv", [H], F32, kind="ExternalInput")
    gamma = nc.dram_tensor("gamma", [H], F32, kind="ExternalInput")
    beta = nc.dram_tensor("beta", [H], F32, kind="ExternalInput")
    out = nc.dram_tensor("out", [S, H], F32, kind="ExternalOutput")
    tens = (x, wq, bq, wk, bk, wv, bv, gamma, beta, out)

    with ExitStack() as ctx:
        tc = ctx.enter_context(tile.TileContext(nc))
        if with_tail is None:
            with_tail = (reps == 1)
        if reps == 1:
            emit_body(nc, tc, ctx, tens, with_tail=with_tail, parts=parts)
        else:
            hints = (mybir.EngineType.PE, mybir.EngineType.DVE,
                     mybir.EngineType.Activation, mybir.EngineType.SP)
            with tc.For_i(0, reps, 1, hint_engines=hints):
                emit_body(nc, tc, ctx, tens, with_tail=False, parts=parts)
    nc.compile()
    return nc


_CACHED_NC = None


def kernel(**inputs):
    global _CACHED_NC
    x_full = np.ascontiguousarray(np.asarray(inputs["inputs"], dtype=np.float32))
    args = {k: np.ascontiguousarray(np.asarray(inputs[k], dtype=np.float32))
            for k in ("wq", "bq", "wk", "bk", "wv", "bv", "gamma", "beta")}
    if _CACHED_NC is None:
        _CACHED_NC = build_nc(reps=1)
    nc = _CACHED_NC
    in_maps = []
    for b in range(N_CORES):
        m = {"x": x_full[b]}
        m.update(args)
        in_maps.append(m)
    res = run_bass_kernel_spmd(nc, in_maps, list(range(N_CORES)))
    out = np.stack([res.results[b]["out"] for b in range(N_CORES)], axis=0)
    return out.astype(np.float32)


# revision 10
# speedup vs baseline: 1.1773x; 1.0278x over previous
"""Trainium2 Bass kernel for nn_MultiHeadAttention_23467701305746.

Reference computation (batch 8, seq 1024, hidden 512, 16 heads x 32):
  q/k/v = relu(x @ W + b); scores = q k^T / sqrt(32); attn = softmax(scores)
  out = attn @ v + x;  BatchNorm1d over (batch, seq) per channel, eps=1e-3.

Sharding: data-parallel over batch, 1 batch element per NeuronCore (8 cores).
BatchNorm batch statistics are combined with a tiny (4 KB) AllReduce.

The kernel is softmax-exp bound: 16.8M exp elements/core = 131072/lane.
ACT runs exp at 1 elem/cycle/lane (1.2 GHz), so exp alone would be ~110us.
v2 splits exp across TWO engines working on alternating score chunks:
  - ACT: true exp (PSUM f32 -> SBUF bf16), (FD+222)/1.2 ns per chunk.
  - DVE: one-op Schraudolph: bf16_bits = round_i16(s * A7 + B7) with
    A7 = log2(e)/sqrt(32)*128, B7 ~ 16249.5. The f32->i16 output convert
    rounds to nearest (hardware-verified), so a single tensor_scalar
    (mult, add) from PSUM produces the bf16 exp approximation directly.
    Elementwise error +-3.3%; softmax ratio cancellation keeps the final
    output error ~3e-4 (verified in numpy bit-exact sim).
Remaining elementwise work balances: relu epilogues / U evacuations split
ACT/DVE by knobs; residual add, square (BN variance), and the final BN
scale/shift run on GPSIMD; weight f32->bf16 casts ride on gpsimd SWDGE
DMA (descriptor cast). Per head pair (2 heads row/col-packed per PE pass):
  S^T[k,q] = kT_h^T qT_h (row-tiled x2); exp chunk [128,2,512] -> bf16
  One 33-col matmul per head gives U^T rows AND the softmax denominator
  row in a single pass (stationary = [v_h | 1]).
U^T chunks --PE bf16 transpose--> layout A; strided reciprocal + broadcast
multiply normalizes; residual add + BN partial sums (ones-matmul)
-> AllReduce -> scale/shift (Quake rsqrt on DVE; no ACT table swap) -> out.
"""

import math
import numpy as np
from contextlib import ExitStack

import concourse.bass as bass
import concourse.tile as tile
from concourse import bacc, mybir
from concourse.bass_utils import run_bass_kernel_spmd
from concourse.masks import make_identity

F32 = mybir.dt.float32
F32R = mybir.dt.float32r
BF16 = mybir.dt.bfloat16
I32 = mybir.dt.int32
I16 = mybir.dt.int16
OP = mybir.AluOpType
AF = mybir.ActivationFunctionType

N_CORES = 8
S = 1024          # sequence length per core (= per batch element)
H = 512           # hidden
NH = 16           # heads
D = 32            # head dim
KC = H // 128     # 4 contraction chunks over hidden
SC = S // 128     # 8 chunks over sequence
QH = S // 512     # 2 query halves (N=512 fp32 matmul limit)
BN_EPS = 1e-3
INV_SQRT_D = 1.0 / math.sqrt(D)
N_ROWS_TOTAL = 8 * S  # BN stats denominator (batch*seq)
QUAKE_C = 0x5F3759DF

# Schraudolph exp constants (bf16 bit pattern = round(s*A7 + B7))
EXP_A7 = float(math.log2(math.e) / math.sqrt(D) * 128.0)
EXP_B7 = 16249.5

# ---- balance knobs ----
EXP_ACT_NUM = 66      # score chunks of 128 assigned to ACT (rest DVE)
XT_EVAC_ACT = 8       # of the 8 xT evac chunks, how many on ACT (rest DVE)
US_EVAC_ACT = 8       # of the 8 U evacuations, how many on ACT (rest DVE)
V_RELU_DVE = False    # v-projection relu epilogue on DVE instead of ACT
RESID_GPS = True      # residual add on GPSIMD
SQ_GPS = True         # BN square on GPSIMD
T2_GPS = True         # BN scale/shift on GPSIMD
CONVW_GPS_DMA = True  # weight f32->bf16 converts via gpsimd dma cast


def _exp_engine(chunk_idx):
    """Deterministic ACT/DVE interleave with EXP_ACT_NUM/128 chunks on ACT,
    spread evenly so the engines alternate rather than phase-separate."""
    pos = chunk_idx % 128
    acc_before = (pos * EXP_ACT_NUM) // 128
    acc_after = ((pos + 1) * EXP_ACT_NUM) // 128
    return "act" if acc_after > acc_before else "dve"


def emit_body(nc, tc, outer_ctx, tens, with_tail=True, parts=("prep", "attn")):
    x, wq, bq, wk, bk, wv, bv, gamma, beta, out = tens
    ctx = outer_ctx.enter_context(ExitStack())

    const = ctx.enter_context(tc.tile_pool(name="const", bufs=1))
    xpool = ctx.enter_context(tc.tile_pool(name="xpool", bufs=1))
    qkp = ctx.enter_context(tc.tile_pool(name="qkp", bufs=1))
    vpool = ctx.enter_context(tc.tile_pool(name="vpool", bufs=1))
    statp = ctx.enter_context(tc.tile_pool(name="statp", bufs=1))
    psum = ctx.enter_context(tc.tile_pool(name="psum", bufs=1, space="PSUM"))

    # ---- constants ----
    ident = const.tile([128, 128], F32, tag="ident", name="ident")
    make_identity(nc, ident[:, :])
    ident_bf = const.tile([128, 128], BF16, tag="ident_bf", name="ident_bf")
    nc.vector.tensor_copy(ident_bf[:, :], ident[:, :])
    ones_f32 = const.tile([128, 512], F32, tag="ones_f32", name="ones_f32")
    nc.vector.memset(ones_f32[:, :], 1.0)
    exp_warm = const.tile([1, 1], F32, tag="exp_warm", name="exp_warm")
    nc.scalar.activation(exp_warm[:, :], ones_f32[0:1, 0:1], AF.Exp)
    warm_mv = const.tile([128, 512], BF16, tag="warm_mv", name="warm_mv")
    nc.vector.memset(warm_mv[:, :], 0.0)
    wps = psum.tile([128, 512], F32, tag="score_ps", name="tps", bufs=3)
    for _ in range(10):
        nc.tensor.matmul(wps[:, :], ident_bf[:, :], warm_mv[:, :],
                         start=True, stop=True)
    ones_row_b = const.tile([1, 128], BF16, tag="ones_row_b", name="ones_row_b")  # K=1 lhsT
    nc.vector.tensor_copy(ones_row_b[:, :], ones_f32[0:1, 0:128])
    ones_row_r = const.tile([1, 128], F32R, tag="ones_row_r", name="ones_row_r")
    nc.vector.tensor_copy(ones_row_r[:, :], ones_f32[0:1, 0:128])
    ones_col_r = const.tile([128, 1], F32R, tag="ones_col_r", name="ones_col_r")
    nc.vector.tensor_copy(ones_col_r[:, :], ones_f32[:, 0:1])
    gamma_sb = const.tile([1, 512], F32, tag="gamma", name="gamma")
    beta_sb = const.tile([1, 512], F32, tag="beta", name="beta")

    # ---- x load (strided DMAs so transposes start early) ----
    x_sb = xpool.tile([128, SC, 512], F32, tag="x_sb", name="x_sb")
    xr = x[:, :].rearrange("(r p) c -> p r c", p=128)
    for q4 in range(4):
        lo, hi = q4 * (SC // 4), (q4 + 1) * (SC // 4)
        nc.sync.dma_start(x_sb[:, lo:hi, :], xr[:, lo:hi, :])

    qT = [qkp.tile([128, S], BF16, tag=f"qT{c}", name=f"qT{c}") for c in range(KC)]
    kT = [qkp.tile([128, S], BF16, tag=f"kT{c}", name=f"kT{c}") for c in range(KC)]
    # v is stored as [128k, head, 33]: columns 0:32 are v_h, column 32 is 1.0
    # so one U matmul per head yields U rows plus the softmax denominator row
    vbig = vpool.tile([128, SC, NH, 33], BF16, tag="vbig", name="vbig")
    v_r = [vbig[:, kv] for kv in range(SC)]

    if "prep" not in parts:
        nc.sync.dma_start(gamma_sb[:, :], gamma[:].unsqueeze(0))
        nc.sync.dma_start(beta_sb[:, :], beta[:].unsqueeze(0))
        # bisection mode: fill qT/kT/v with memset instead of real projections
        for c in range(KC):
            nc.vector.memset(qT[c][:, :], 0.01)
            nc.vector.memset(kT[c][:, :], 0.01)
        nc.vector.memset(vbig[:, :, :, :], 0.5)
        nc.vector.tensor_copy(
            vbig[:, :, :, 32:33],
            ones_f32[:, 0:SC * NH].rearrange("p (a b) -> p a b", a=SC).unsqueeze(3))
    # ---- pools (prep + attention live together for the interleave) ----
    expp = ctx.enter_context(tc.tile_pool(name="expp", bufs=2))
    usb = ctx.enter_context(tc.tile_pool(name="usb", bufs=2))
    outp = ctx.enter_context(tc.tile_pool(name="outp", bufs=1))
    tmpp = ctx.enter_context(tc.tile_pool(name="tmpp", bufs=2))
    outA = outp.tile([128, SC, 512], F32R, tag="outA", name="outA")

    have_prep = "prep" in parts
    have_attn = "attn" in parts
    exp_chunk_counter = [0]

    if have_prep:
        wpool = ctx.enter_context(tc.tile_pool(name="wpool", bufs=1))
        wstg = ctx.enter_context(tc.tile_pool(name="wstg", bufs=1))
        xtp = ctx.enter_context(tc.tile_pool(name="xtp", bufs=1))

        # one strided DMA per weight matrix into f32 staging [128p, kc, 512c]
        # (p = contraction sub-index), then one cast to bf16
        w_st, w_r = {}, {}
        for name, wt in (("q", wq), ("k", wk), ("v", wv)):
            st = wstg.tile([128, KC, 512], F32, tag=f"ws{name}", name=f"ws{name}")
            nc.sync.dma_start(st[:, :, :],
                              wt[:, :].rearrange("(k p) c -> p k c", p=128))
            w_st[name] = st
            w_r[name] = wpool.tile([128, KC, 512], BF16, tag=f"w{name}",
                                   name=f"w{name}")
        bqk = wpool.tile([128, 2, KC], F32, tag="bqk", name="bqk")
        nc.sync.dma_start(bqk[:, 0, :], bq[:].rearrange("(k p) -> p k", p=128))
        nc.sync.dma_start(bqk[:, 1, :], bk[:].rearrange("(k p) -> p k", p=128))
        bv_sb = wpool.tile([1, 512], F32, tag="bv_sb", name="bv_sb")
        nc.sync.dma_start(bv_sb[:, :], bv[:].unsqueeze(0))
        nc.sync.dma_start(gamma_sb[:, :], gamma[:].unsqueeze(0))
        nc.sync.dma_start(beta_sb[:, :], beta[:].unsqueeze(0))
        bv_b = wpool.tile([1, 512], BF16, tag="bv_b", name="bv_b")
        nc.vector.tensor_copy(bv_b[:, :], bv_sb[:, :])

        def conv_w(name):
            if CONVW_GPS_DMA:
                nc.gpsimd.dma_start(
                    w_r[name][:, :, :].rearrange("p a b -> p (a b)"),
                    w_st[name][:, :, :].rearrange("p a b -> p (a b)"))
            else:
                nc.vector.tensor_copy(
                    w_r[name][:, :, :].rearrange("p a b -> p (a b)"),
                    w_st[name][:, :, :].rearrange("p a b -> p (a b)"))

        # q/k weight converts first (they gate the first projections)
        conv_w("q")
        conv_w("k")

        # x transpose -> xT (bf16); PSUM evacuation split ACT/DVE
        xT = [xtp.tile([128, S], BF16, tag=f"xT{c}", name=f"xT{c}") for c in range(KC)]
        for c in range(KC):
            for half in range(2):
                tp4 = psum.tile([128, 4, 128], F32, tag="score_ps",
                                name="tps", bufs=3)
                for rr in range(4):
                    r = half * 4 + rr
                    nc.tensor.transpose(tp4[:, rr, :],
                                        x_sb[:, r, c * 128:(c + 1) * 128],
                                        ident[:, :])
                dst = xT[c][:, half * 512:(half + 1) * 512]
                src = tp4[:, :, :].rearrange("p a b -> p (a b)")
                if c * 2 + half < XT_EVAC_ACT:
                    nc.scalar.copy(dst, src)
                else:
                    nc.vector.tensor_copy(dst, src)

        def proj_qk(oc):
            # projections q,k chunk oc (layout B); both seq halves share one
            # 2-bank slot so the relu+bias epilogue batches to FD=1024
            for wi, (wkey, dest) in enumerate((("q", qT), ("k", kT))):
                ps = psum.tile([128, 2, 512], F32, tag="score_ps",
                               name="tps", bufs=3)
                for sh in range(QH):
                    for kc in range(KC):
                        nc.tensor.matmul(
                            ps[:, sh, :],
                            w_r[wkey][:, kc, oc * 128:(oc + 1) * 128],
                            xT[kc][:, sh * 512:(sh + 1) * 512],
                            start=(kc == 0), stop=(kc == KC - 1))
                nc.scalar.activation(
                    dest[oc][:, :],
                    ps[:, :, :].rearrange("p a c -> p (a c)"),
                    AF.Relu, bias=bqk[:, wi, oc:oc + 1], scale=1.0)

        def proj_v():
            # v (layout A); bias via ones-matmul; 2 kv chunks share one slot
            # so the relu epilogue batches to FD=1024
            conv_w("v")
            nc.vector.tensor_copy(
                vbig[:, :, :, 32:33],
                ones_f32[:, 0:SC * NH].rearrange("p (a b) -> p a b", a=SC).unsqueeze(3))
            for kv2 in range(SC // 2):
                ps = psum.tile([128, 2, 512], F32, tag="score_ps",
                               name="tps", bufs=3)
                for g in range(2):
                    kv = kv2 * 2 + g
                    for kc in range(KC):
                        nc.tensor.matmul(ps[:, g, :],
                                         xT[kc][:, kv * 128:(kv + 1) * 128],
                                         w_r["v"][:, kc, :],
                                         start=(kc == 0), stop=False)
                    nc.tensor.matmul(ps[:, g, :], ones_row_b[:, :], bv_b[:, :],
                                     start=False, stop=True)
                dst = vbig[:, kv2 * 2:kv2 * 2 + 2, :, 0:32]
                srcv = ps[:, :, :].rearrange("p g (h d) -> p g h d", h=NH)
                if V_RELU_DVE:
                    nc.vector.tensor_scalar(out=dst, in0=srcv, scalar1=0.0,
                                            scalar2=None, op0=OP.max)
                else:
                    nc.scalar.activation(dst, srcv, AF.Relu)

    def emit_pair_mm(hp):
        h0, h1 = 2 * hp, 2 * hp + 1
        ch = h0 // 4                 # qT/kT chunk holding these heads
        p0 = (h0 % 4) * 32           # partition base of h0 within chunk
        p1 = (h1 % 4) * 32
        ups = psum.tile([128, S], F32, tag="u_ps", name="u_ps")

        def emit_scores(kc, qh):
            # half-size chunk with a double-buffered PSUM tag so the PE can
            # run one chunk ahead of the exp engines instead of stalling
            sps = psum.tile([128, 2, 512], F32, tag="score_ps",
                            name="score_ps", bufs=3)
            for j, pb in ((0, p0), (1, p1)):
                nc.tensor.matmul(
                    sps[:, j, :],
                    kT[ch][pb:pb + 32, kc * 128:(kc + 1) * 128],
                    qT[ch][pb:pb + 32, qh * 512:(qh + 1) * 512],
                    start=True, stop=True, tile_position=(pb, 0))
            ex = expp.tile([128, 2, 512], BF16, tag="expS", name="expS",
                           bufs=4)
            # alternate exp between ACT (true exp) and DVE (Schraudolph)
            if _exp_engine(exp_chunk_counter[0]) == "act":
                nc.scalar.activation(
                    ex[:, :, :].rearrange("p a c -> p (a c)"),
                    sps[:, :, :].rearrange("p a c -> p (a c)"),
                    AF.Exp, scale=INV_SQRT_D)
            else:
                nc.vector.tensor_scalar(
                    out=ex[:, :, :].bitcast(I16), in0=sps[:, :, :],
                    scalar1=EXP_A7, scalar2=EXP_B7,
                    op0=OP.mult, op1=OP.add)
            exp_chunk_counter[0] += 1
            return ex

        def emit_u(kc, qh, ex):
            # one 33-col matmul per head: rows 0:32 = U_h, row 32 = rowsum
            st, sp = (kc == 0), (kc == SC - 1)
            q0, q1 = qh * 512, (qh + 1) * 512
            nc.tensor.matmul(ups[0:33, q0:q1], v_r[kc][:, h0, :],
                             ex[:, 0, :], start=st, stop=sp,
                             tile_position=(0, 0))
            nc.tensor.matmul(ups[64:97, q0:q1], v_r[kc][:, h1, :],
                             ex[:, 1, :], start=st, stop=sp,
                             tile_position=(0, 64))

        chunks = [(kc, qh) for kc in range(SC) for qh in range(QH)]
        prev_c, prev_ex = chunks[0], emit_scores(*chunks[0])
        for cur in chunks[1:]:
            cur_ex = emit_scores(*cur)
            emit_u(prev_c[0], prev_c[1], prev_ex)
            prev_c, prev_ex = cur, cur_ex
        emit_u(prev_c[0], prev_c[1], prev_ex)

        # evacuate U (rows [0:32]=U_h0, [32]=rowsum_h0, [64:96]=U_h1,
        # [96]=rowsum_h1) to bf16 SBUF; transposes happen in _fin
        us = usb.tile([128, S], BF16, tag="u_sb", name="u_sb")
        if hp < US_EVAC_ACT:
            nc.scalar.copy(us[:, :], ups[:, :])
        else:
            nc.vector.tensor_copy(us[:, :], ups[:, :])
        return us

    sq = tmpp.tile([128, SC, 512], F32R, tag="sq", name="sq", bufs=1)

    def emit_pair_fin(hp, us):
        # transpose each seq chunk to layout A and divide by rowsum column;
        # all 8 bf16 transposes of the pair pack into one PSUM bank tile
        tpb = psum.tile([128, SC, 128], BF16, tag="score_ps", name="tps",
                        bufs=3)
        for sc in range(SC):
            nc.tensor.transpose(tpb[:, sc, :], us[:, sc * 128:(sc + 1) * 128],
                                ident_bf[:, :])
        rsr = statp.tile([128, SC, 2], F32, tag="rsr", name="rsr", bufs=2)
        # one strided reciprocal for both rowsum columns, one fused
        # broadcast-multiply for both heads (4D strided views)
        nc.vector.reciprocal(rsr[:, :, 0:2], tpb[:, :, 32:97:64])
        in0 = tpb[:, :, :].rearrange("p s (b c) -> p s b c", b=4)[:, :, 0:3:2, :]
        ov = outA[:, :, 64 * hp:64 * (hp + 1)].rearrange(
            "p s (b c) -> p s b c", b=2)
        in0b, in1b = bass.broadcast_tensor_aps(in0, rsr[:, :, 0:2].unsqueeze(3))
        nc.vector.tensor_mul(ov, in0b, in1b)
        # residual + BN square for this pair's 64 output columns, on GPSIMD
        # (otherwise idle) so the serial tail after the last pair shrinks
        c0, c1 = 64 * hp, 64 * (hp + 1)
        if RESID_GPS:
            nc.gpsimd.tensor_add(outA[:, :, c0:c1], outA[:, :, c0:c1],
                                 x_sb[:, :, c0:c1])
        else:
            nc.vector.tensor_add(outA[:, :, c0:c1], outA[:, :, c0:c1],
                                 x_sb[:, :, c0:c1])
        if SQ_GPS:
            nc.gpsimd.tensor_mul(sq[:, :, c0:c1], outA[:, :, c0:c1],
                                 outA[:, :, c0:c1])
        else:
            nc.vector.tensor_mul(sq[:, :, c0:c1], outA[:, :, c0:c1],
                                 outA[:, :, c0:c1])

    # interleaved emission: projections feed attention pairs chunk by chunk
    # so PE projection work hides under exp work (per-engine queues are
    # in-order; emission order controls overlap). Each pair's evac
    # transposes are deferred past the next projection chunk so the PE
    # doesn't idle waiting on the us-copy.
    if have_prep and have_attn:
        proj_qk(0)
        proj_v()
        usd = {}
        usd[0] = emit_pair_mm(0)
        proj_qk(1)
        usd[1] = emit_pair_mm(1)
        emit_pair_fin(0, usd[0])
        proj_qk(2)
        usd[2] = emit_pair_mm(2)
        emit_pair_fin(1, usd[1])
        proj_qk(3)
        usd[3] = emit_pair_mm(3)
        emit_pair_fin(2, usd[2])
        for hp in range(4, NH // 2):
            usd[hp] = emit_pair_mm(hp)
            emit_pair_fin(hp - 1, usd[hp - 1])
        emit_pair_fin(NH // 2 - 1, usd[NH // 2 - 1])
    elif have_prep:
        for oc in range(KC):
            proj_qk(oc)
        proj_v()
    elif have_attn:
        prev = emit_pair_mm(0)
        for hp in range(1, NH // 2):
            nxt = emit_pair_mm(hp)
            emit_pair_fin(hp - 1, prev)
            prev = nxt
        emit_pair_fin(NH // 2 - 1, prev)

    if not have_attn:
        for sc in range(SC):
            nc.vector.tensor_scalar(out=outA[:, sc, :], in0=ones_f32[:, :],
                                    scalar1=0.1, scalar2=None, op0=OP.mult)
    if not have_attn:
        nc.vector.tensor_copy(sq[:, :, :], outA[:, :, :])
    # ---- tail: BN stats + AllReduce, scale/shift, output ----
    # (residual + square already happened per pair inside emit_pair_fin)
    sum_ps = psum.tile([1, 512], F32, tag="score_ps", name="sum_ps", bufs=3)
    sq_ps = psum.tile([1, 512], F32, tag="u_ps", name="sq_ps")
    for sc in range(SC):
        nc.tensor.matmul(sum_ps[:, :], ones_col_r, outA[:, sc, :],
                         start=(sc == 0), stop=(sc == SC - 1))
        nc.tensor.matmul(sq_ps[:, :], ones_col_r, sq[:, sc, :],
                         start=(sc == 0), stop=(sc == SC - 1))

    stats_sb = statp.tile([1, 1024], F32, tag="stats_sb", name="stats_sb")
    nc.vector.tensor_copy(stats_sb[:, 0:512], sum_ps[:, :])
    nc.vector.tensor_copy(stats_sb[:, 512:1024], sq_ps[:, :])
    if with_tail:
        dram = ctx.enter_context(tc.tile_pool(name="dram", bufs=1, space="DRAM"))
        cc_in = dram.tile([1, 1024], F32)
        cc_out = dram.tile([1, 1024], F32)
        nc.sync.dma_start(cc_in[:, :], stats_sb[:, :])
        nc.gpsimd.collective_compute(
            "AllReduce", OP.add,
            replica_groups=[list(range(N_CORES))],
            ins=[cc_in[:, :].opt()], outs=[cc_out[:, :].opt()])
        gstats = statp.tile([1, 1024], F32, tag="gstats", name="gstats")
        nc.sync.dma_start(gstats[:, :], cc_out[:, :])
    else:
        # timing-only build: skip collective (banned in control flow) but run
        # the full tail math on local stats so engine load is representative
        gstats = stats_sb
    mean = statp.tile([1, 512], F32, tag="mean", name="mean")
    nc.vector.tensor_scalar(out=mean[:, :], in0=gstats[:, 0:512],
                            scalar1=1.0 / N_ROWS_TOTAL, scalar2=None,
                            op0=OP.mult)
    esq = statp.tile([1, 512], F32, tag="esq", name="esq")
    nc.vector.tensor_scalar(out=esq[:, :], in0=gstats[:, 512:1024],
                            scalar1=1.0 / N_ROWS_TOTAL, scalar2=None,
                            op0=OP.mult)
    var = statp.tile([1, 512], F32, tag="var", name="var")
    nc.vector.tensor_mul(var[:, :], mean[:, :], mean[:, :])
    nc.vector.tensor_sub(var[:, :], esq[:, :], var[:, :])
    # ve = var + eps; y = rsqrt(ve) via Quake bit-trick + 1 Newton step
    # (keeps the whole tail off ACT so the exp table set never swaps)
    ve = statp.tile([1, 512], F32, tag="ve", name="ve")
    nc.vector.tensor_scalar(out=ve[:, :], in0=var[:, :], scalar1=BN_EPS,
                            scalar2=None, op0=OP.add)
    ybits = statp.tile([1, 512], I32, tag="ybits", name="ybits")
    nc.vector.tensor_scalar(out=ybits[:, :], in0=ve[:, :].bitcast(I32),
                            scalar1=1, scalar2=None,
                            op0=OP.arith_shift_right)
    nc.vector.tensor_scalar(out=ybits[:, :], in0=ybits[:, :],
                            scalar1=-1, scalar2=None, op0=OP.bitwise_xor)
    y = statp.tile([1, 512], F32, tag="y", name="y")
    nc.vector.tensor_scalar(out=y[:, :].bitcast(I32), in0=ybits[:, :],
                            scalar1=QUAKE_C + 1, scalar2=None, op0=OP.add)
    t = statp.tile([1, 512], F32, tag="t", name="t")
    for _ in range(1):
        nc.vector.tensor_mul(t[:, :], y[:, :], y[:, :])
        nc.vector.tensor_mul(t[:, :], t[:, :], ve[:, :])
        nc.vector.tensor_scalar(out=t[:, :], in0=t[:, :], scalar1=-0.5,
                                scalar2=1.5, op0=OP.mult, op1=OP.add)
        nc.vector.tensor_mul(y[:, :], y[:, :], t[:, :])
    A = statp.tile([1, 512], F32R, tag="A", name="A")
    nc.vector.tensor_mul(A[:, :], y[:, :], gamma_sb[:, :])
    B = statp.tile([1, 512], F32R, tag="Bt", name="Bt")
    nc.vector.tensor_mul(B[:, :], mean[:, :], A[:, :])
    nc.vector.tensor_sub(B[:, :], beta_sb[:, :], B[:, :])
    a_ps = psum.tile([128, 512], F32, tag="score_ps", name="a_ps", bufs=3)
    b_ps = psum.tile([128, 512], F32, tag="u_ps", name="b_ps")
    nc.tensor.matmul(a_ps[:, :], ones_row_r, A[:, :],
                     start=True, stop=True)
    nc.tensor.matmul(b_ps[:, :], ones_row_r, B[:, :],
                     start=True, stop=True)
    t2 = tmpp.tile([128, SC, 512], F32, tag="t2", name="t2", bufs=1)
    outr = out[:, :].rearrange("(r p) c -> p r c", p=128)
    if T2_GPS:
        # split scale/shift between GPSIMD (needs SBUF-staged A/B) and DVE
        # (reads the PSUM broadcast directly) to halve the serial tail
        a_sb = statp.tile([128, 512], F32, tag="a_sb", name="a_sb")
        b_sb = statp.tile([128, 512], F32, tag="b_sb", name="b_sb")
        nc.vector.tensor_copy(a_sb[:, :], a_ps[:, :])
        nc.vector.tensor_copy(b_sb[:, :], b_ps[:, :])
        for g in range(4):
            lo, hi = g * 2, (g + 1) * 2
            r0, r1 = lo, lo + 1  # r0 -> GPS, r1 -> DVE, concurrent
            nc.gpsimd.tensor_mul(t2[:, r0, :], outA[:, r0, :], a_sb[:, :])
            in0, in1 = bass.broadcast_tensor_aps(outA[:, r1:r1 + 1, :],
                                                 a_ps[:, :].unsqueeze(1))
            nc.vector.tensor_mul(t2[:, r1:r1 + 1, :], in0, in1)
            nc.gpsimd.tensor_add(t2[:, r0, :], t2[:, r0, :], b_sb[:, :])
            in0, in1 = bass.broadcast_tensor_aps(t2[:, r1:r1 + 1, :],
                                                 b_ps[:, :].unsqueeze(1))
            nc.vector.tensor_add(t2[:, r1:r1 + 1, :], in0, in1)
            nc.sync.dma_start(outr[:, lo:hi, :], t2[:, lo:hi, :])
    else:
        hh = SC // 2
        for h0, h1 in ((0, hh), (hh, SC)):
            in0, in1 = bass.broadcast_tensor_aps(outA[:, h0:h1, :],
                                                 a_ps[:, :].unsqueeze(1))
            nc.vector.tensor_mul(t2[:, h0:h1, :], in0, in1)
            in0, in1 = bass.broadcast_tensor_aps(t2[:, h0:h1, :],
                                                 b_ps[:, :].unsqueeze(1))
            nc.vector.tensor_add(t2[:, h0:h1, :], in0, in1)
            nc.sync.dma_start(outr[:, h0:h1, :], t2[:, h0:h1, :])
    ctx.close()


def build_nc(reps=1, parts=("prep", "attn"), with_tail=None):
    nc = bacc.Bacc("TRN2", target_bir_lowering=False, debug=False)
    x = nc.dram_tensor("x", [S, H], F32, kind="ExternalInput")
    wq = nc.dram_tensor("wq", [H, H], F32, kind="ExternalInput")
    bq = nc.dram_tensor("bq", [H], F32, kind="ExternalInput")
    wk = nc.dram_tensor("wk", [H, H], F32, kind="ExternalInput")
    bk = nc.dram_tensor("bk", [H], F32, kind="ExternalInput")
    wv = nc.dram_tensor("wv", [H, H], F32, kind="ExternalInput")
    bv = nc.dram_tensor("bv", [H], F32, kind="ExternalInput")
    gamma = nc.dram_tensor("gamma", [H], F32, kind="ExternalInput")
    beta = nc.dram_tensor("beta", [H], F32, kind="ExternalInput")
    out = nc.dram_tensor("out", [S, H], F32, kind="ExternalOutput")
    tens = (x, wq, bq, wk, bk, wv, bv, gamma, beta, out)

    with ExitStack() as ctx:
        tc = ctx.enter_context(tile.TileContext(nc))
        if with_tail is None:
            with_tail = (reps == 1)
        if reps == 1:
            emit_body(nc, tc, ctx, tens, with_tail=with_tail, parts=parts)
        else:
            hints = (mybir.EngineType.PE, mybir.EngineType.DVE,
                     mybir.EngineType.Activation, mybir.EngineType.SP)
            with tc.For_i(0, reps, 1, hint_engines=hints):
                emit_body(nc, tc, ctx, tens, with_tail=False, parts=parts)
    nc.compile()
    return nc


_CACHED_NC = None


def kernel(**inputs):
    global _CACHED_NC
    x_full = np.ascontiguousarray(np.asarray(inputs["inputs"], dtype=np.float32))
    args = {k: np.ascontiguousarray(np.asarray(inputs[k], dtype=np.float32))
            for k in ("wq", "bq", "wk", "bk", "wv", "bv", "gamma", "beta")}
    if _CACHED_NC is None:
        _CACHED_NC = build_nc(reps=1)
    nc = _CACHED_NC
    in_maps = []
    for b in range(N_CORES):
        m = {"x": x_full[b]}
        m.update(args)
        in_maps.append(m)
    res = run_bass_kernel_spmd(nc, in_maps, list(range(N_CORES)))
    out = np.stack([res.results[b]["out"] for b in range(N_CORES)], axis=0)
    return out.astype(np.float32)
